# revision 9
# baseline (speedup 1.0000x reference)
"""Trainium2 Bass kernel for nn_CausalGraphReasoning.

Reference computation (n=64 nodes, d=128, h=256):
  causal_matrix[i,j]   = sigmoid(MLP_cd(concat(x_i, x_j)))       masked i!=j
  confounder[i,j,k]    = sigmoid(MLP_cf(concat(x_i, x_j, x_k)))  masked distinct
  modified_features    = x with row `node` replaced by MLP_ip(concat(x_node, v))

Key algebraic restructure: the first-layer matmuls over concatenated features
decompose into per-node projections, e.g.
  MLP_cf layer1(i,j,k) = relu(A[i] + B[j] + C[k] + b1)
with A = x@W1[0:d], B = x@W1[d:2d], C = x@W1[2d:3d].  That turns the O(n^3 *
3d*h) triplet matmul into O(n*d*h) projections plus O(n^3*h) broadcast-add-relu
and an O(n^3*h) dot with W2 — ~100x less FLOPs.

Sharding: the i-axis of the triplet (and pair) grid is split across the 8
cores (8 i-values each).  Each core receives the full node set (for j/k) plus
only its own i-rows, computes its [8,64,64] confounder block and [8,64] causal
rows, and the host concatenates the blocks.  Weights are replicated.

On-device layout: hidden channels on partitions (2 half-tiles of 128), batch
in the free dim.  Per k, th_k = relu(D + C[:,k]) is one fused
tensor_scalar(add,max) / activation(Relu, bias) instruction of [128,512],
split between VectorE and ScalarE.  The 256-channel reduction with cfW2 runs
on TensorE as 128 accumulating fp32r matmuls (N=512) whose lhsT is a sliding
[128,64] window over a zero-padded cfW2 column, so row k of a single
[64,512] PSUM bank receives z_k while the other rows accumulate zeros —
avoiding any single-partition evacuation work.
"""

import os
import sys

for _p in ("/opt/trn_rl_repo", "/root/.axon_site", "/root/.axon_site/_ro/trn_rl_repo",
           "/root/.axon_site/_ro/pypackages"):
    if os.path.isdir(_p) and _p not in sys.path:
        sys.path.append(_p)

import numpy as np

N = 64          # nodes
D = 128         # input dim
H = 256         # hidden dim
NCORES = 8
IB = N // NCORES   # i-rows per core = 8
P = IB * N         # pairs per core = 512

_F32 = None  # mybir.dt.float32, set lazily
_PROGRAM = None  # (nc, names) cache


def _build_program():
    import concourse.bacc as bacc
    import concourse.mybir as mybir
    import concourse.tile as tile

    f32 = mybir.dt.float32
    f32r = mybir.dt.float32r
    f16 = mybir.dt.float16
    AF = mybir.ActivationFunctionType
    OP = mybir.AluOpType

    nc = bacc.Bacc("TRN2", target_bir_lowering=False, debug=False,
                   num_devices=NCORES)

    # ---- DRAM I/O (inputs merged into 5 tensors to cut DMA triggers) ----
    # big f32 [128, 852]: 0:64 xT | 64:72 xTi | 72:328 wip1 | 328:584 wip2 |
    #   584:592 bias stack | 592:593 ival(row0) | 593:595 sb | 595:596 xnodeT |
    #   596:852 row0 = ipW1[128,:] (wip1b)
    d_big = nc.dram_tensor("big", [D, 852], f32, kind="ExternalInput")
    # 10 stacked [128,128] lhsT blocks: cfA0 cfA1 cfB0 cfB1 cfC0 cfC1
    #                                   cdA0 cdA1 cdB0 cdB1
    d_w1s = nc.dram_tensor("w1s", [D, 10 * 128], f32, kind="ExternalInput")
    # cdW2 k-halves [128,128]x2 + cdW3 col (fp32r for 1cyc/row matmul)
    d_w2sr = nc.dram_tensor("w2sr", [D, 257], f32r, kind="ExternalInput")
    # sliding-window padded cfW2 halves (fp16): per-half 192-col region,
    # zeros except col 64 (h0) / col 256 (h1) = cfW2 half
    d_w2p = nc.dram_tensor("w2p", [D, 384], f16, kind="ExternalInput")
    # rows 0:64 triplet mask (k-major), row 64 pair mask
    d_trimp = nc.dram_tensor("trimp", [N + 1, P], f32, kind="ExternalInput")

    d_conf = nc.dram_tensor("conf_out", [N, P], f32, kind="ExternalOutput")
    d_caus = nc.dram_tensor("causal_out", [1, P], f32, kind="ExternalOutput")
    d_eff = nc.dram_tensor("eff_out", [D, 1], f32, kind="ExternalOutput")

    with tile.TileContext(nc) as tc:
        with (
            tc.tile_pool(name="const", bufs=1) as cpool,
            tc.tile_pool(name="work", bufs=1) as wpool,
            tc.tile_pool(name="th", bufs=12) as thpool,
            tc.tile_pool(name="ps1", bufs=3, space="PSUM") as ps1,
            tc.tile_pool(name="psz", bufs=1, space="PSUM") as psz,
            tc.tile_pool(name="psp", bufs=2, space="PSUM") as psp,
        ):
            # ---- load constants (2 queues: sync + gpsimd) ------------
            big = cpool.tile([D, 852], f32, name="big_sb")
            w1s = cpool.tile([D, 10 * 128], f32, name="w1s_sb")
            w2sr = cpool.tile([D, 257], f32r, name="w2sr_sb")
            w2p = cpool.tile([D, 384], f16, name="w2p_sb")
            trimm = cpool.tile([N, P], f32, name="trimm_sb")
            pairm = cpool.tile([1, P], f32, name="pairm_sb")
            nc.sync.dma_start(big[:], d_big[:])
            nc.sync.dma_start(w1s[:], d_w1s[:])
            nc.gpsimd.dma_start(w2p[:], d_w2p[:])
            nc.gpsimd.dma_start(w2sr[:], d_w2sr[:])
            nc.gpsimd.dma_start(trimm[:], d_trimp[0:N, :])
            nc.gpsimd.dma_start(pairm[:], d_trimp[N:N + 1, :])

            xT = big[:, 0:64]
            xTi = big[:, 64:72]
            wip1 = big[:, 72:328]
            wip2 = big[:, 328:584]
            ival = big[0:1, 592:593]
            xnodeT = big[:, 595:596]
            wip1b = big[0:1, 596:852]

            def bias(i):  # bias stack column i (0..7)
                return big[:, 584 + i:585 + i]

            def w1blk(i):
                return w1s[:, i * 128:(i + 1) * 128]

            # ---- stage 1: per-node projections (channels on partitions)
            Bh, Ch, Ah, PAh, PBh = [], [], [], [], []
            for h in range(2):
                pB = ps1.tile([D, N], f32, name=f"pB{h}", tag="ps1")
                nc.tensor.matmul(pB[:], w1blk(2 + h), xT, start=True, stop=True)
                t = wpool.tile([D, N], f16, name=f"Bh{h}")
                nc.scalar.copy(t[:], pB[:])
                Bh.append(t)

                pC = ps1.tile([D, N], f32, name=f"pC{h}", tag="ps1")
                nc.tensor.matmul(pC[:], w1blk(4 + h), xT, start=True, stop=True)
                t = wpool.tile([D, N], f32, name=f"Ch{h}")
                nc.scalar.copy(t[:], pC[:])
                Ch.append(t)

                pA = ps1.tile([D, N], f32, name=f"pA{h}", tag="ps1")
                nc.tensor.matmul(pA[:, :IB], w1blk(0 + h), xTi, start=True, stop=True)
                t = wpool.tile([D, IB], f32, name=f"Ah{h}")
                nc.scalar.activation(t[:], pA[:, :IB], AF.Identity, bias=bias(h))
                Ah.append(t)

                pPA = ps1.tile([D, N], f32, name=f"pPA{h}", tag="ps1")
                nc.tensor.matmul(pPA[:, :IB], w1blk(6 + h), xTi, start=True, stop=True)
                t = wpool.tile([D, IB], f32, name=f"PAh{h}")
                nc.scalar.activation(t[:], pPA[:, :IB], AF.Identity, bias=bias(2 + h))
                PAh.append(t)

                pPB = ps1.tile([D, N], f32, name=f"pPB{h}", tag="ps1")
                nc.tensor.matmul(pPB[:], w1blk(8 + h), xT, start=True, stop=True)
                t = wpool.tile([D, N], f32, name=f"PBh{h}")
                nc.scalar.copy(t[:], pPB[:])
                PBh.append(t)

            # ---- stage 2: D = A'[i] + B[j] (fp16)  and PH1 = relu(...) f32r
            Dh, PH1h = [], []
            for h in range(2):
                Dt = wpool.tile([D, P], f16, name=f"Dh{h}")
                Pt = wpool.tile([D, P], f32r, name=f"PH1h{h}")
                for di in range(IB):
                    s = slice(di * N, (di + 1) * N)
                    nc.vector.tensor_scalar(
                        Dt[:, s], Bh[h][:], Ah[h][:, di:di + 1], None, OP.add)
                    nc.vector.tensor_scalar(
                        Pt[:, s], PBh[h][:], PAh[h][:, di:di + 1], 0.0,
                        OP.add, OP.max)
                Dh.append(Dt)
                PH1h.append(Pt)

            # ---- intervention head (tiny; runs early on idle engines) --
            ih = []
            for h in range(2):
                c = slice(h * 128, (h + 1) * 128)
                pIh = ps1.tile([D, N], f32, name=f"pI{h}", tag="ps1")
                nc.tensor.matmul(pIh[:, 0:1], wip1[:, c], xnodeT,
                                 start=True, stop=False)
                nc.tensor.matmul(pIh[:, 0:1], wip1b[:, c], ival,
                                 start=False, stop=True)
                t = wpool.tile([D, 1], f32, name=f"ih{h}")
                nc.scalar.activation(t[:], pIh[:, 0:1], AF.Relu, bias=bias(5 + h))
                ih.append(t)
            pE = ps1.tile([D, N], f32, name="pE", tag="ps1")
            nc.tensor.matmul(pE[:, 0:1], wip2[:, 0:128], ih[0][:],
                             start=True, stop=False)
            nc.tensor.matmul(pE[:, 0:1], wip2[:, 128:256], ih[1][:],
                             start=False, stop=True)
            eff = wpool.tile([D, 1], f32, name="eff")
            nc.scalar.activation(eff[:], pE[:, 0:1], AF.Identity, bias=bias(7))
            nc.sync.dma_start(d_eff[:], eff[:])

            # ---- pair MLP layers 2+3 --------------------------------
            pP = psp.tile([D, 512], f32, name="pP", tag="psp")
            nc.tensor.matmul(pP[:], w2sr[:, 0:128], PH1h[0][:],
                             start=True, stop=False)
            nc.tensor.matmul(pP[:], w2sr[:, 128:256], PH1h[1][:],
                             start=False, stop=True)
            ph2 = wpool.tile([D, P], f32r, name="ph2")
            nc.scalar.activation(ph2[:], pP[:], AF.Relu, bias=bias(4))
            pCz = psp.tile([D, 512], f32, name="pCz", tag="psp")
            nc.tensor.matmul(pCz[0:1, :], w2sr[:, 256:257], ph2[:],
                             start=True, stop=True)
            caus = wpool.tile([1, P], f32, name="caus")
            nc.scalar.activation(caus[:], pCz[0:1, :], AF.Sigmoid,
                                 bias=big[0:1, 594:595])
            nc.vector.tensor_tensor(caus[:], caus[:], pairm[:], OP.mult)
            nc.sync.dma_start(d_caus[:], caus[:])

            # ---- main triplet loop ----------------------------------
            # th = relu(D + C[:,k]) per half (fp16) -> z rows accumulate in
            # TWO alternating PSUM banks (so consecutive matmul drains
            # overlap), each via the sliding-window lhsT over w2p.
            zA = psz.tile([D, 512], f32, name="zA")
            zB = psz.tile([D, 512], f32, name="zB")
            for k in range(N):
                for h in range(2):
                    idx = 2 * k + h
                    th = thpool.tile([D, P], f16, name="th", tag="th")
                    if idx % 4 == 3:
                        nc.scalar.activation(th[:], Dh[h][:], AF.Relu,
                                             bias=Ch[h][:, k:k + 1])
                    else:
                        nc.vector.tensor_scalar(
                            th[:], Dh[h][:], Ch[h][:, k:k + 1], 0.0,
                            OP.add, OP.max)
                    lhs = w2p[:, 64 + 192 * h - k: 192 + 192 * h - k]
                    tgt = zA if idx % 2 == 0 else zB
                    nc.tensor.matmul(tgt[:], lhs, th[:],
                                     start=(idx < 2), stop=(idx >= 2 * N - 2))

            zBsb = wpool.tile([N, P], f32, name="zBsb")
            nc.scalar.copy(zBsb[:], zB[0:N, :])
            zsum = wpool.tile([N, P], f32, name="zsum")
            nc.vector.tensor_tensor(zsum[:], zA[0:N, :], zBsb[:], OP.add)
            sig = wpool.tile([N, P], f32, name="sig")
            nc.scalar.activation(sig[:], zsum[:], AF.Sigmoid,
                                 bias=big[0:N, 593:594])
            nc.vector.tensor_tensor(sig[:], sig[:], trimm[:], OP.mult)
            nc.sync.dma_start(d_conf[:], sig[:])

    nc.compile()
    return nc


def _get_program():
    global _PROGRAM
    if _PROGRAM is None:
        _PROGRAM = _build_program()
    return _PROGRAM


def _prep_inputs(inputs):
    """Host-side sharding/layout prep -> list of 8 per-core input dicts."""
    x = np.ascontiguousarray(np.asarray(inputs["node_features"], np.float32))
    node = int(np.asarray(inputs["intervention_node"]))
    ivalv = float(np.asarray(inputs["intervention_value"]).reshape(-1)[0])
    cdW1 = np.asarray(inputs["cdW1"], np.float32)
    cdb1 = np.asarray(inputs["cdb1"], np.float32)
    cdW2 = np.asarray(inputs["cdW2"], np.float32)
    cdb2 = np.asarray(inputs["cdb2"], np.float32)
    cdW3 = np.asarray(inputs["cdW3"], np.float32)
    cdb3 = np.asarray(inputs["cdb3"], np.float32)
    cfW1 = np.asarray(inputs["cfW1"], np.float32)
    cfb1 = np.asarray(inputs["cfb1"], np.float32)
    cfW2 = np.asarray(inputs["cfW2"], np.float32)
    cfb2 = np.asarray(inputs["cfb2"], np.float32)
    ipW1 = np.asarray(inputs["ipW1"], np.float32)
    ipb1 = np.asarray(inputs["ipb1"], np.float32)
    ipW2 = np.asarray(inputs["ipW2"], np.float32)
    ipb2 = np.asarray(inputs["ipb2"], np.float32)

    xT = np.ascontiguousarray(x.T)                       # [D, N]
    big = np.zeros((D, 852), np.float32)
    big[:, 0:64] = xT
    big[:, 72:328] = ipW1[0:128]
    big[:, 328:456] = ipW2[0:128]
    big[:, 456:584] = ipW2[128:256]
    big[:, 584] = cfb1[0:128]
    big[:, 585] = cfb1[128:256]
    big[:, 586] = cdb1[0:128]
    big[:, 587] = cdb1[128:256]
    big[:, 588] = cdb2
    big[:, 589] = ipb1[0:128]
    big[:, 590] = ipb1[128:256]
    big[:, 591] = ipb2
    big[0, 592] = ivalv
    big[:, 593] = float(cfb2[0])
    big[:, 594] = float(cdb3[0])
    big[:, 595] = x[node]
    big[0, 596:852] = ipW1[128]

    w1s = np.concatenate([cfW1[0:128], cfW1[128:256], cfW1[256:384],
                          cdW1[0:128], cdW1[128:256]], axis=1)  # [128, 1280]
    w2sr = np.zeros((D, 257), np.float32)
    w2sr[:, 0:128] = cdW2[0:128]
    w2sr[:, 128:256] = cdW2[128:256]
    w2sr[:, 256] = cdW3[:, 0]
    w2p = np.zeros((D, 384), np.float16)
    w2p[:, 64] = cfW2[0:128, 0].astype(np.float16)
    w2p[:, 256] = cfW2[128:256, 0].astype(np.float16)

    idx = np.arange(N)
    in_maps = []
    for m in range(NCORES):
        i0 = m * IB
        ii = idx[i0:i0 + IB]
        bigm = big.copy()
        bigm[:, 64:72] = x[i0:i0 + IB].T
        # trim[k, di*64 + j] = 1 if (i0+di, j, k) pairwise distinct
        i_g = ii[None, :, None]
        j_g = idx[None, None, :]
        k_g = idx[:, None, None]
        trim = ((i_g != j_g) & (j_g != k_g) & (i_g != k_g)).astype(np.float32)
        trimp = np.zeros((N + 1, P), np.float32)
        trimp[0:N] = trim.reshape(N, P)
        trimp[N] = (ii[:, None] != idx[None, :]).astype(np.float32).reshape(P)
        in_maps.append({
            "big": bigm, "w1s": w1s, "w2sr": w2sr, "w2p": w2p, "trimp": trimp,
        })
    return in_maps, x, node


def _run(inputs, trace=False):
    from concourse.bass_utils import run_bass_kernel_spmd

    nc = _get_program()
    in_maps, x, node = _prep_inputs(inputs)
    res = run_bass_kernel_spmd(nc, in_maps, core_ids=list(range(NCORES)),
                               trace=trace)

    causal = np.zeros((N, N), np.float32)
    conf = np.zeros((N, N, N), np.float32)
    for m in range(NCORES):
        i0 = m * IB
        causal[i0:i0 + IB] = res.results[m]["causal_out"].reshape(IB, N)
        co = res.results[m]["conf_out"]                  # [N(k), P]
        conf[i0:i0 + IB] = co.reshape(N, IB, N).transpose(1, 2, 0)
    modified = x.copy()
    modified[node] = res.results[0]["eff_out"][:, 0]
    return (causal, conf, modified), res


def kernel(**inputs):
    outs, _ = _run(inputs, trace=False)
    return outs
